# revision 1
# baseline (speedup 1.0000x reference)
"""Product-quantizer VQ kernel for Trainium2 (8 NeuronCores, data-parallel).

Problem (hardcoded): x (32, 512, 2048) f32, codebooks (4, 1024, 128) f32.
Returns (out (32,512,2048) f32, loss f32, perplexity f32) matching

    x_flat = transpose(x, (0,2,1)).reshape(-1, 512); groups of 128 channels
    dist   = |xg|^2 - 2 xg.cb + |cb|^2 ; code = argmin_k dist
    out    = codebook[code] (straight-through value), loss = 1.25*mean((xq-x)^2)
    perplexity = mean_g exp(-sum p log(p + 1e-10)), p = counts/n_tok

Device strategy (per core = 4 images, 8192 tokens):
  - argmin_k dist == argmax_k s, s = x.cb - |cb|^2/2. s is computed at full
    fp32 precision on the PE with THREE fp32r matmuls (x and cb split into
    exact hi/lo fp32r pairs; fp32r = fp32 with 11-bit mantissa, so hi+lo is
    exact and every partial product is exact in fp32 PSUM accumulation), plus
    one contraction-2 matmul adding the bias rows [bh; bl].
  - DVE max + max_index over PSUM [tok=128, K=1024] give m and code index.
  - Pool adds g*1024 to the index; indirect DMA gathers codebook rows;
    PE transposes [tok, Dg] -> [Dg, tok]; ACT copies PSUM->SBUF; DMA stores.
  - loss via identity: sum min_dist = sum(x^2) - 2*sum(m)  (sum(x^2) on host,
    sum(m) accumulated on DVE per partition lane).
  - counts for perplexity: host bincount of the returned code indices.
"""
import numpy as np

import concourse.bass as bass
import concourse.mybir as mybir
import concourse.tile as tile
from concourse import bacc
from concourse.bass_utils import run_bass_kernel_spmd

# problem constants
N, C, T = 32, 512, 2048
G, K, DG = 4, 1024, 128
NCORES = 8
NPC = N // NCORES          # images per core
TILES_T = T // 128         # 16 token-tiles per image
NTILES = NPC * TILES_T     # 64 tiles per core
BETA = 0.25
EPS_LOG = 1e-10

_cache = {}


def _round_fp32r(x):
    u = np.ascontiguousarray(x).view(np.uint32)
    lsb = ((u >> 12) & 1).astype(np.uint32)
    u2 = (u + np.uint32(0x7FF) + lsb) & np.uint32(0xFFFFF000)
    return u2.view(np.float32)


def _build_program():
    nc = bacc.Bacc("TRN2", target_bir_lowering=False, debug=False)
    f32, f32r, u32, bf16 = (mybir.dt.float32, mybir.dt.float32r,
                            mybir.dt.uint32, mybir.dt.bfloat16)

    t_xsp = nc.dram_tensor("xsp", [NPC, DG, 2, G, T], f32r, kind="ExternalInput")
    t_cbTh = nc.dram_tensor("cbTh", [DG, G * K], f32r, kind="ExternalInput")
    t_cbTl = nc.dram_tensor("cbTl", [DG, G * K], f32r, kind="ExternalInput")
    t_bsp = nc.dram_tensor("bsp", [2, G * K], f32r, kind="ExternalInput")
    t_ones2 = nc.dram_tensor("ones2", [2, 128], f32r, kind="ExternalInput")
    t_cb = nc.dram_tensor("cbflat", [G * K, DG], f32, kind="ExternalInput")
    t_ident = nc.dram_tensor("ident", [128, 128], f32, kind="ExternalInput")

    t_out = nc.dram_tensor("out", [NPC, C, T], f32, kind="ExternalOutput")
    t_idx = nc.dram_tensor("idx", [NTILES, 128, G], u32, kind="ExternalOutput")
    t_macc = nc.dram_tensor("macc", [128, 1], f32, kind="ExternalOutput")

    with tile.TileContext(nc) as tc:
        with tc.tile_pool(name="const", bufs=1) as cpool, \
             tc.tile_pool(name="xin", bufs=3) as xpool, \
             tc.tile_pool(name="small", bufs=4) as spool, \
             tc.tile_pool(name="gat", bufs=3) as gpool, \
             tc.tile_pool(name="ps", bufs=2, space="PSUM") as ps, \
             tc.tile_pool(name="psf", bufs=1, space="PSUM") as psf:

            cbTh = cpool.tile([DG, G * K], f32r)
            cbTl = cpool.tile([DG, G * K], f32r)
            bsp = cpool.tile([2, G * K], f32r)
            ones2 = cpool.tile([2, 128], f32r)
            ident = cpool.tile([128, 128], f32)
            macc = cpool.tile([128, 1], f32)

            nc.sync.dma_start(cbTh[:], t_cbTh[:, :])
            nc.sync.dma_start(cbTl[:], t_cbTl[:, :])
            nc.sync.dma_start(bsp[:], t_bsp[:, :])
            nc.sync.dma_start(ones2[:], t_ones2[:, :])
            nc.sync.dma_start(ident[:], t_ident[:, :])
            nc.vector.memset(macc[:], 0.0)

            fence_ps = psf.tile([8, 8], f32)
            for lhs, rhs in ((cbTh, cbTl), (ones2, bsp), (ident, ident)):
                nc.tensor.matmul(out=fence_ps[0:1, 0:1],
                                 lhsT=lhs[:].bitcast(bf16)[0:1, 0:1],
                                 rhs=rhs[:].bitcast(bf16)[0:1, 0:1],
                                 start=True, stop=True)

            for n in range(NPC):
                for ti in range(TILES_T):
                    tile_id = n * TILES_T + ti
                    t0 = ti * 128

                    xhl = xpool.tile([DG, 2, G, 128], f32r, tag="xhl")
                    nc.sync.dma_start(xhl[:], t_xsp[n, :, :, :, t0:t0 + 128])
                    # PE observes the x DMA via a cheap fence matmul
                    nc.tensor.matmul(out=fence_ps[0:1, 0:1],
                                     lhsT=xhl[:].bitcast(bf16)[0:1, 0:1, 0:1, 0:1],
                                     rhs=ident[:].bitcast(bf16)[0:1, 0:1],
                                     start=True, stop=True)

                    coll = spool.tile([128, G], u32, tag="coll")

                    for g in range(G):
                        xh_g = xhl[:, 0, g, :]
                        xl_g = xhl[:, 1, g, :]

                        s_psum = ps.tile([128, K], f32, tag="s")
                        # WAR fence: absorbs wait on this PSUM buf's last reader
                        nc.tensor.matmul(out=s_psum[0:1, 0:1],
                                         lhsT=ident[:].bitcast(bf16)[0:1, 0:1],
                                         rhs=ident[:].bitcast(bf16)[0:1, 0:1],
                                         start=True, stop=True)
                        for kc in (0, 512):
                            cslice = slice(g * K + kc, g * K + kc + 512)
                            pslice = slice(kc, kc + 512)
                            nc.tensor.matmul(out=s_psum[:, pslice], lhsT=xh_g,
                                             rhs=cbTh[:, cslice], start=True, stop=False)
                            nc.tensor.matmul(out=s_psum[:, pslice], lhsT=xh_g,
                                             rhs=cbTl[:, cslice], start=False, stop=False)
                            nc.tensor.matmul(out=s_psum[:, pslice], lhsT=xl_g,
                                             rhs=cbTh[:, cslice], start=False, stop=False)
                            nc.tensor.matmul(out=s_psum[:, pslice], lhsT=ones2[:],
                                             rhs=bsp[:, cslice], start=False, stop=True)

                        m8 = spool.tile([128, 8], f32, tag="m8")
                        idx8 = spool.tile([128, 8], u32, tag="idx8")
                        nc.vector.max(out=m8[:], in_=s_psum[:])
                        nc.vector.max_index(out=idx8[:], in_max=m8[:], in_values=s_psum[:])
                        nc.vector.tensor_tensor(out=macc[:], in0=macc[:],
                                                in1=m8[:, 0:1], op=mybir.AluOpType.add)

                        nc.gpsimd.tensor_scalar(
                            out=coll[:, g:g + 1], in0=idx8[:, 0:1],
                            scalar1=g * K, scalar2=None, op0=mybir.AluOpType.add)

                        xq = gpool.tile([128, DG], f32, tag="xq")
                        nc.gpsimd.indirect_dma_start(
                            out=xq[:], out_offset=None, in_=t_cb[:, :],
                            in_offset=bass.IndirectOffsetOnAxis(ap=coll[:, g:g + 1], axis=0))

                        xqT_psum = ps.tile([DG, 128], f32, tag="xqT")
                        nc.tensor.matmul(out=xqT_psum[0:1, 0:1],
                                         lhsT=ident[:].bitcast(bf16)[0:1, 0:1],
                                         rhs=ident[:].bitcast(bf16)[0:1, 0:1],
                                         start=True, stop=True)
                        nc.tensor.transpose(out=xqT_psum[:], in_=xq[:], identity=ident[:])
                        xqT = gpool.tile([DG, 128], f32, tag="xqTs")
                        nc.scalar.copy(out=xqT[:], in_=xqT_psum[:])
                        nc.gpsimd.dma_start(
                            out=t_out[n, g * DG:(g + 1) * DG, t0:t0 + 128], in_=xqT[:])

                    nc.gpsimd.dma_start(out=t_idx[tile_id, :, :], in_=coll[:])

            nc.gpsimd.dma_start(out=t_macc[:, :], in_=macc[:])

    nc.compile()
    return nc


def _prep_inputs(x, codebooks):
    x = np.ascontiguousarray(x, dtype=np.float32)
    cb = np.ascontiguousarray(codebooks, dtype=np.float32)

    xh = _round_fp32r(x)
    xl = x - xh
    # [N, C, T] -> [N, 4, 128, T] -> [N, 128, 4, T]; stack hi/lo -> [N,128,2,4,T]
    def arr(a):
        return a.reshape(N, G, DG, T).transpose(0, 2, 1, 3)
    xsp = np.stack([arr(xh), arr(xl)], axis=2)  # [N, 128, 2, 4, T]
    xsp = np.ascontiguousarray(xsp, dtype=np.float32)

    cbT = cb.transpose(0, 2, 1).reshape(G, DG, K)      # [G, Dg, K]
    cbT_full = np.concatenate([cbT[g] for g in range(G)], axis=1)  # [Dg, G*K]
    cbTh = _round_fp32r(np.ascontiguousarray(cbT_full))
    cbTl = np.ascontiguousarray(cbT_full - cbTh)

    b = (-0.5 * (cb.astype(np.float64) ** 2).sum(-1)).astype(np.float32)  # [G, K]
    b = b.reshape(G * K)
    bh = _round_fp32r(b)
    bl = b - bh
    bsp = np.ascontiguousarray(np.stack([bh, bl]))       # [2, G*K]

    consts = {
        "cbTh": cbTh, "cbTl": cbTl, "bsp": bsp,
        "ones2": np.ones((2, 128), dtype=np.float32),
        "cbflat": np.ascontiguousarray(cb.reshape(G * K, DG)),
        "ident": np.eye(128, dtype=np.float32),
    }
    in_maps = []
    for c in range(NCORES):
        m = {"xsp": np.ascontiguousarray(xsp[c * NPC:(c + 1) * NPC])}
        m.update(consts)
        in_maps.append(m)
    return in_maps


def run_device(x, codebooks, trace=False):
    """Returns (results list per core, BassKernelResults)."""
    if "nc" not in _cache:
        _cache["nc"] = _build_program()
    nc = _cache["nc"]
    in_maps = _prep_inputs(x, codebooks)
    res = run_bass_kernel_spmd(nc, in_maps, core_ids=list(range(NCORES)),
                               trace=trace)
    return res


def kernel(x, codebooks):
    x = np.asarray(x, dtype=np.float32)
    codebooks = np.asarray(codebooks, dtype=np.float32)
    res = run_device(x, codebooks)

    outs, idxs, maccs = [], [], []
    for r in res.results:
        outs.append(r["out"])
        idxs.append(r["idx"])
        maccs.append(r["macc"])
    out = np.concatenate(outs, axis=0)              # (32, 512, 2048)

    # loss = 1.25 * sum(min_dist) / (n_tok * C);  sum(min_dist) = sum x^2 - 2 sum m
    sum_x2 = float((x.astype(np.float64) ** 2).sum())
    sum_m = float(sum(m.astype(np.float64).sum() for m in maccs))
    n_tok = N * T
    loss = (1.0 + BETA) * (sum_x2 - 2.0 * sum_m) / (n_tok * C)

    # perplexity from code histogram
    all_idx = np.concatenate([i.reshape(-1) for i in idxs])
    counts = np.bincount(all_idx, minlength=G * K).astype(np.float64).reshape(G, K)
    probs = counts / n_tok
    perp = np.exp(-(probs * np.log(probs + EPS_LOG)).sum(axis=-1))
    perplexity = perp.mean()

    return out, np.float32(loss), np.float32(perplexity)


# revision 9
# speedup vs baseline: 2.0237x; 2.0237x over previous
"""Product-quantizer VQ kernel for Trainium2 (8 NeuronCores, data-parallel).

Problem (hardcoded): x (32, 512, 2048) f32, codebooks (4, 1024, 128) f32.
Returns (out (32,512,2048) f32, loss f32, perplexity f32) matching

    x_flat = transpose(x, (0,2,1)).reshape(-1, 512); groups of 128 channels
    dist   = |xg|^2 - 2 xg.cb + |cb|^2 ; code = argmin_k dist
    out    = codebook[code] (straight-through value), loss = 1.25*mean((xq-x)^2)
    perplexity = mean_g exp(-sum p log(p + 1e-10)), p = counts/n_tok

Device strategy (per core = 4 images, 8192 tokens):
  - argmin_k dist == argmax_k s, s = x.cb - |cb|^2/2. s is computed at full
    fp32 precision on the PE with THREE fp32r matmuls (x and cb split into
    exact hi/lo fp32r pairs; fp32r = fp32 with 11-bit mantissa, so hi+lo is
    exact and every partial product is exact in fp32 PSUM accumulation), plus
    one contraction-2 matmul adding the bias rows [bh; bl].
  - DVE max + max_index over PSUM [tok=128, K=1024] give m and code index.
  - Pool adds g*1024 to the index; indirect DMA gathers codebook rows;
    PE transposes [tok, Dg] -> [Dg, tok]; ACT copies PSUM->SBUF; DMA stores.
  - loss via identity: sum min_dist = sum(x^2) - 2*sum(m)  (sum(x^2) on host,
    sum(m) accumulated on DVE per partition lane).
  - counts for perplexity: host bincount of the returned code indices.
"""
import numpy as np

import concourse.bass as bass
import concourse.mybir as mybir
import concourse.tile as tile
from concourse import bacc
from concourse.bass_utils import run_bass_kernel_spmd

# problem constants
N, C, T = 32, 512, 2048
G, K, DG = 4, 1024, 128
NCORES = 8
NPC = N // NCORES          # images per core
TILES_T = T // 128         # 16 token-tiles per image
NTILES = NPC * TILES_T     # 64 tiles per core
BETA = 0.25
EPS_LOG = 1e-10

_cache = {}


def _round_fp32r(x):
    u = np.ascontiguousarray(x).view(np.uint32)
    lsb = ((u >> 12) & 1).astype(np.uint32)
    u2 = (u + np.uint32(0x7FF) + lsb) & np.uint32(0xFFFFF000)
    return u2.view(np.float32)


def _build_program(store_hw=True, pair_stationary=True, swdge_queues=4,
                   skip_dve=False, skip_gather=False, skip_mm=False, defer=True,
                   act_idx=True):
    nc = bacc.Bacc("TRN2", target_bir_lowering=False, debug=False,
               num_swdge_queues=swdge_queues)
    f32, f32r, u32, bf16 = (mybir.dt.float32, mybir.dt.float32r,
                            mybir.dt.uint32, mybir.dt.bfloat16)

    t_xsp = nc.dram_tensor("xsp", [NPC, DG, 2, G, T], f32r, kind="ExternalInput")
    t_cbTh = nc.dram_tensor("cbTh", [DG, G * K], f32r, kind="ExternalInput")
    t_cbTl = nc.dram_tensor("cbTl", [DG, G * K], f32r, kind="ExternalInput")
    t_bsp = nc.dram_tensor("bsp", [2, G * K], f32r, kind="ExternalInput")
    t_ones2 = nc.dram_tensor("ones2", [2, 128], f32r, kind="ExternalInput")
    t_cb = nc.dram_tensor("cbflat", [G * K, DG], f32, kind="ExternalInput")
    t_ident = nc.dram_tensor("ident", [128, 128], f32, kind="ExternalInput")
    t_iota = nc.dram_tensor("iota", [128, K], mybir.dt.float16, kind="ExternalInput")

    t_out = nc.dram_tensor("out", [NPC, C, T], f32, kind="ExternalOutput")
    t_idx = nc.dram_tensor("idx", [NTILES, 128, G], u32, kind="ExternalOutput")
    t_macc = nc.dram_tensor("macc", [128, 1], f32, kind="ExternalOutput")

    with tile.TileContext(nc) as tc:
        with tc.tile_pool(name="const", bufs=1) as cpool, \
             tc.tile_pool(name="xin", bufs=3) as xpool, \
             tc.tile_pool(name="small", bufs=4) as spool, \
             tc.tile_pool(name="gat", bufs=10) as gpool, \
             tc.tile_pool(name="ps", bufs=2, space="PSUM") as ps, \
             tc.tile_pool(name="psf", bufs=1, space="PSUM") as psf:

            cbTh = cpool.tile([DG, G * K], f32r)
            cbTl = cpool.tile([DG, G * K], f32r)
            bsp = cpool.tile([2, G * K], f32r)
            ones2 = cpool.tile([2, 128], f32r)
            ident = cpool.tile([128, 128], f32)
            iota = cpool.tile([128, K], mybir.dt.float16)
            macc = cpool.tile([128, 1], f32)

            nc.sync.dma_start(cbTh[:], t_cbTh[:, :])
            nc.sync.dma_start(cbTl[:], t_cbTl[:, :])
            nc.sync.dma_start(bsp[:], t_bsp[:, :])
            nc.sync.dma_start(ones2[:], t_ones2[:, :])
            nc.sync.dma_start(ident[:], t_ident[:, :])
            nc.sync.dma_start(iota[:], t_iota[:, :])
            nc.vector.memset(macc[:], 0.0)

            fence_ps = psf.tile([8, 8], f32)
            for lhs, rhs in ((cbTh, cbTl), (ones2, bsp), (ident, ident)):
                nc.tensor.matmul(out=fence_ps[0:1, 0:1],
                                 lhsT=lhs[:].bitcast(bf16)[0:1, 0:1],
                                 rhs=rhs[:].bitcast(bf16)[0:1, 0:1],
                                 start=True, stop=True)

            pending = []   # deferred output work: (xq_tile, n, g, t0)

            def flush_pending():
                for (xq_, n_, g_, t0_) in pending:
                    xqT_psum = ps.tile([DG, 128], f32, tag="xqT")
                    nc.tensor.matmul(out=xqT_psum[0:1, 0:1],
                                     lhsT=ident[:].bitcast(bf16)[0:1, 0:1],
                                     rhs=ident[:].bitcast(bf16)[0:1, 0:1],
                                     start=True, stop=True)
                    nc.tensor.transpose(out=xqT_psum[:], in_=xq_[:], identity=ident[:])
                    xqT = gpool.tile([DG, 128], f32, tag="xqTs")
                    nc.scalar.copy(out=xqT[:], in_=xqT_psum[:])
                    st = nc.sync if store_hw else nc.gpsimd
                    st.dma_start(
                        out=t_out[n_, g_ * DG:(g_ + 1) * DG, t0_:t0_ + 128], in_=xqT[:])
                pending.clear()

            for n in range(NPC):
                for ti in range(TILES_T):
                    tile_id = n * TILES_T + ti
                    t0 = ti * 128

                    xhl = xpool.tile([DG, 2, G, 128], f32r, tag="xhl")
                    nc.sync.dma_start(xhl[:], t_xsp[n, :, :, :, t0:t0 + 128])
                    # PE observes the x DMA via a cheap fence matmul
                    nc.tensor.matmul(out=fence_ps[0:1, 0:1],
                                     lhsT=xhl[:].bitcast(bf16)[0:1, 0:1, 0:1, 0:1],
                                     rhs=ident[:].bitcast(bf16)[0:1, 0:1],
                                     start=True, stop=True)

                    coll = spool.tile([128, G], u32, tag="coll")

                    for g in range(G):
                        xh_g = xhl[:, 0, g, :]
                        xl_g = xhl[:, 1, g, :]

                        s_psum = ps.tile([128, K], f32, tag="s")
                        # WAR fence: absorbs wait on this PSUM buf's last reader
                        nc.tensor.matmul(out=s_psum[0:1, 0:1],
                                         lhsT=ident[:].bitcast(bf16)[0:1, 0:1],
                                         rhs=ident[:].bitcast(bf16)[0:1, 0:1],
                                         start=True, stop=True)
                        def CS(kc):
                            return slice(g * K + kc, g * K + kc + 512)
                        def PS(kc):
                            return slice(kc, kc + 512)
                        if skip_mm:
                            nc.tensor.matmul(out=s_psum[:, PS(0)], lhsT=xh_g,
                                             rhs=cbTh[:, CS(0)], start=True, stop=True)
                            nc.tensor.matmul(out=s_psum[:, PS(512)], lhsT=xh_g,
                                             rhs=cbTh[:, CS(512)], start=True, stop=True)
                        elif pair_stationary:
                            nc.tensor.matmul(out=s_psum[:, PS(0)], lhsT=xh_g,
                                             rhs=cbTh[:, CS(0)], start=True, stop=False)
                            nc.tensor.matmul(out=s_psum[:, PS(512)], lhsT=xh_g,
                                             rhs=cbTh[:, CS(512)], start=True, stop=False)
                            nc.tensor.matmul(out=s_psum[:, PS(0)], lhsT=xh_g,
                                             rhs=cbTl[:, CS(0)], start=False, stop=False)
                            nc.tensor.matmul(out=s_psum[:, PS(512)], lhsT=xh_g,
                                             rhs=cbTl[:, CS(512)], start=False, stop=False)
                            nc.tensor.matmul(out=s_psum[:, PS(0)], lhsT=xl_g,
                                             rhs=cbTh[:, CS(0)], start=False, stop=False)
                            nc.tensor.matmul(out=s_psum[:, PS(512)], lhsT=xl_g,
                                             rhs=cbTh[:, CS(512)], start=False, stop=False)
                            nc.tensor.matmul(out=s_psum[:, PS(0)], lhsT=ones2[:],
                                             rhs=bsp[:, CS(0)], start=False, stop=True)
                            nc.tensor.matmul(out=s_psum[:, PS(512)], lhsT=ones2[:],
                                             rhs=bsp[:, CS(512)], start=False, stop=True)
                        else:
                            for kc in (0, 512):
                                nc.tensor.matmul(out=s_psum[:, PS(kc)], lhsT=xh_g,
                                                 rhs=cbTh[:, CS(kc)], start=True, stop=False)
                                nc.tensor.matmul(out=s_psum[:, PS(kc)], lhsT=xh_g,
                                                 rhs=cbTl[:, CS(kc)], start=False, stop=False)
                                nc.tensor.matmul(out=s_psum[:, PS(kc)], lhsT=xl_g,
                                                 rhs=cbTh[:, CS(kc)], start=False, stop=False)
                                nc.tensor.matmul(out=s_psum[:, PS(kc)], lhsT=ones2[:],
                                                 rhs=bsp[:, CS(kc)], start=False, stop=True)

                        m8 = spool.tile([128, 8], f32, tag="m8")
                        idx8 = spool.tile([128, 8], u32, tag="idx8")
                        if skip_dve:
                            nc.vector.max(out=m8[:], in_=s_psum[:, 0:8])
                            nc.vector.max_index(out=idx8[:], in_max=m8[:],
                                                in_values=s_psum[:, 0:8])
                        elif act_idx:
                            nc.vector.max(out=m8[:], in_=s_psum[:])
                        else:
                            nc.vector.max(out=m8[:], in_=s_psum[:])
                            nc.vector.max_index(out=idx8[:], in_max=m8[:],
                                                in_values=s_psum[:])
                        nc.vector.tensor_tensor(out=macc[:], in0=macc[:],
                                                in1=m8[:, 0:1], op=mybir.AluOpType.add)

                        if act_idx:
                            anti = spool.tile([128, K], mybir.dt.float16, tag="anti")
                            nc.scalar.activation(
                                out=anti[:], in_=s_psum[:],
                                func=mybir.ActivationFunctionType.Sign,
                                bias=m8[:, 0:1], scale=-1.0)
                            junk = spool.tile([128, K], mybir.dt.float16, tag="junk")
                            nc.vector.tensor_tensor(out=junk[:], in0=anti[:],
                                                    in1=iota[:], op=mybir.AluOpType.mult)
                            dotv = spool.tile([128, 1], f32, tag="dotv")
                            nc.scalar.activation(
                                out=junk[:], in_=junk[:],
                                func=mybir.ActivationFunctionType.Identity,
                                accum_out=dotv[:])
                            nc.gpsimd.tensor_scalar(
                                out=coll[:, g:g + 1], in0=dotv[:, 0:1],
                                scalar1=float(K * (K - 1) // 2 + g * K),
                                scalar2=-1.0,
                                op0=mybir.AluOpType.subtract,
                                op1=mybir.AluOpType.mult)
                        else:
                            nc.gpsimd.tensor_scalar(
                                out=coll[:, g:g + 1], in0=idx8[:, 0:1],
                                scalar1=g * K, scalar2=None, op0=mybir.AluOpType.add)

                        xq = gpool.tile([128, DG], f32, tag="xq")
                        if skip_gather:
                            nc.vector.memset(xq[:, 0:8], 0.0)
                        else:
                            nc.gpsimd.indirect_dma_start(
                                out=xq[:], out_offset=None, in_=t_cb[:, :],
                                in_offset=bass.IndirectOffsetOnAxis(ap=coll[:, g:g + 1], axis=0),
                                bounds_check=G * K - 1, oob_is_err=False)
                        pending.append((xq, n, g, t0))

                    (nc.sync if store_hw else nc.gpsimd).dma_start(
                        out=t_idx[tile_id, :, :], in_=coll[:])
                    # flush the PREVIOUS tile's output path now (one-tile delay)
                    lim = 8 if defer else 4
                    if len(pending) >= lim:
                        prev, rest = pending[:4], pending[4:]
                        pending[:] = prev
                        flush_pending()
                        pending[:] = rest

            flush_pending()
            (nc.sync if store_hw else nc.gpsimd).dma_start(out=t_macc[:, :], in_=macc[:])

    nc.compile()
    return nc


def _prep_inputs(x, codebooks):
    x = np.ascontiguousarray(x, dtype=np.float32)
    cb = np.ascontiguousarray(codebooks, dtype=np.float32)

    xh = _round_fp32r(x)
    xl = x - xh
    # [N, C, T] -> [N, 4, 128, T] -> [N, 128, 4, T]; stack hi/lo -> [N,128,2,4,T]
    def arr(a):
        return a.reshape(N, G, DG, T).transpose(0, 2, 1, 3)
    xsp = np.stack([arr(xh), arr(xl)], axis=2)  # [N, 128, 2, 4, T]
    xsp = np.ascontiguousarray(xsp, dtype=np.float32)

    cbT = cb.transpose(0, 2, 1).reshape(G, DG, K)      # [G, Dg, K]
    cbT_full = np.concatenate([cbT[g] for g in range(G)], axis=1)  # [Dg, G*K]
    cbTh = _round_fp32r(np.ascontiguousarray(cbT_full))
    cbTl = np.ascontiguousarray(cbT_full - cbTh)

    b = (-0.5 * (cb.astype(np.float64) ** 2).sum(-1)).astype(np.float32)  # [G, K]
    b = b.reshape(G * K)
    bh = _round_fp32r(b)
    bl = b - bh
    bsp = np.ascontiguousarray(np.stack([bh, bl]))       # [2, G*K]

    consts = {
        "cbTh": cbTh, "cbTl": cbTl, "bsp": bsp,
        "ones2": np.ones((2, 128), dtype=np.float32),
        "cbflat": np.ascontiguousarray(cb.reshape(G * K, DG)),
        "ident": np.eye(128, dtype=np.float32),
        "iota": np.broadcast_to(np.arange(K, dtype=np.float16), (128, K)).copy(),
    }
    in_maps = []
    for c in range(NCORES):
        m = {"xsp": np.ascontiguousarray(xsp[c * NPC:(c + 1) * NPC])}
        m.update(consts)
        in_maps.append(m)
    return in_maps


def run_device(x, codebooks, trace=False):
    """Returns (results list per core, BassKernelResults)."""
    if "nc" not in _cache:
        _cache["nc"] = _build_program()
    nc = _cache["nc"]
    in_maps = _prep_inputs(x, codebooks)
    res = run_bass_kernel_spmd(nc, in_maps, core_ids=list(range(NCORES)),
                               trace=trace)
    return res


def kernel(x, codebooks):
    x = np.asarray(x, dtype=np.float32)
    codebooks = np.asarray(codebooks, dtype=np.float32)
    res = run_device(x, codebooks)

    outs, idxs, maccs = [], [], []
    for r in res.results:
        outs.append(r["out"])
        idxs.append(r["idx"])
        maccs.append(r["macc"])
    out = np.concatenate(outs, axis=0)              # (32, 512, 2048)

    # loss = 1.25 * sum(min_dist) / (n_tok * C);  sum(min_dist) = sum x^2 - 2 sum m
    sum_x2 = float((x.astype(np.float64) ** 2).sum())
    sum_m = float(sum(m.astype(np.float64).sum() for m in maccs))
    n_tok = N * T
    loss = (1.0 + BETA) * (sum_x2 - 2.0 * sum_m) / (n_tok * C)

    # perplexity from code histogram
    all_idx = np.concatenate([i.reshape(-1) for i in idxs])
    counts = np.bincount(all_idx, minlength=G * K).astype(np.float64).reshape(G, K)
    probs = counts / n_tok
    perp = np.exp(-(probs * np.log(probs + EPS_LOG)).sum(axis=-1))
    perplexity = perp.mean()

    return out, np.float32(loss), np.float32(perplexity)
